# revision 1
# baseline (speedup 1.0000x reference)
"""BrightnessLoss Trainium2 kernel (raw Bass, 8-core data parallel).

reference:
    V(x)   = max_c(clip(x, 0, 1))        over channel dim (RGB)
    result = mean(|V(pred) - V(target)|) over (N, H, W)

Identities used on device:
    clip(max(r,g,b),0,1) == max_c(clip(x,0,1))          (clip is monotone)
    W := relu(1 - relu(m)) == 1 - clip(m, 0, 1)
    |Vp - Vt| == |Wp - Wt|
    sum|Wp - Wt| == 2*sum max(Wp,Wt) - sum Wp - sum Wt

Work is cut into "units" (image chunks along the plane's free dim). Per unit:
    dma pred+targ [128, 3*w] f32  — even units on the SP HWDGE ring, odd
        units on the ACT ring, so one ring's inter-transfer bubble is
        covered by the other ring streaming the next unit
    DVE   m1 = max(R,G); u = (m1 max 0) max B   (fused relu, x2 sides)
    ACT   W = Relu(-u + 1), accum_out = sum(W)  (side sums come free)
    DVE   stt bypass,max: max(Wp,Wt), accum_out = sum
The last image's final chunks shrink (e.g. 1024,768,256) so the post-DMA
dependency chain of the very last unit is short. Partials are written out in
two DMAs (bulk early, last units at the end). Host combines in float64.
"""

import numpy as np

N_CORES = 8
N_IMG = 4  # 32 / 8
C = 3
P = 128
F = 2048  # 512*512 / 128
N_PIX = 32 * 512 * 512
N_CHUNKS = 2  # chunks per plane
TAIL_SPLIT = (768, 256)  # last image final-chunk split (sums to F/N_CHUNKS)


def _plan_units(n_img, f, n_chunks, tail_split):
    """Units: (img, col_offset, width). Last image's final chunk is split
    further per tail_split to shorten the end-of-kernel dependency chain."""
    fc = f // n_chunks
    units = []
    for img in range(n_img):
        offs = [(j * fc, fc) for j in range(n_chunks)]
        if img == n_img - 1 and tail_split:
            assert sum(tail_split) == fc
            off0 = offs[-1][0]
            offs = offs[:-1]
            o = off0
            for w in tail_split:
                offs.append((o, w))
                o += w
        for off, w in offs:
            units.append((img, off, w))
    return units, fc


def _build_program(n_img=N_IMG, f=F, n_chunks=N_CHUNKS, tail_split=TAIL_SPLIT):
    from contextlib import ExitStack

    import concourse.bass as bass
    import concourse.mybir as mybir

    fp32 = mybir.dt.float32
    Alu = mybir.AluOpType
    Act = mybir.ActivationFunctionType

    assert f % n_chunks == 0
    units, fc = _plan_units(n_img, f, n_chunks, tail_split)
    n_units = len(units)

    # detect_race_conditions=False: the raw-mode CoreSim race detector can't
    # see same-engine program-order (DVE m1 -> STT RAW); hardware engines
    # execute in order.
    nc = bass.Bass(
        "TRN2", target_bir_lowering=False, debug=False, detect_race_conditions=False
    )
    pred = nc.dram_tensor("pred", [n_img, C, P, f], fp32, kind="ExternalInput").ap()
    targ = nc.dram_tensor("target", [n_img, C, P, f], fp32, kind="ExternalInput").ap()
    out = nc.dram_tensor(
        "partials", [P, 3 * n_units], fp32, kind="ExternalOutput"
    ).ap()

    with ExitStack() as ctx:
        sb = lambda name, shape: ctx.enter_context(nc.sbuf_tensor(name, shape, fp32))
        sem = lambda name: ctx.enter_context(nc.semaphore(name))

        inb = [[sb(f"in{sl}{s}", [P, C * fc]) for s in range(2)] for sl in range(2)]
        ub = [[sb(f"u{sl}{s}", [P, fc]) for s in range(2)] for sl in range(2)]
        wb = [[sb(f"w{sl}{s}", [P, fc]) for s in range(2)] for sl in range(2)]
        m1 = sb("m1", [P, fc])
        scr = sb("stt_scratch", [P, fc])
        acc = sb("acc", [P, 3 * n_units])

        inp_sem = [sem("inp0"), sem("inp1")]  # pred side, by slot parity
        int_sem = [sem("int0"), sem("int1")]  # targ side, by slot parity
        u_sem = sem("u")
        act_sem = sem("act")
        gp_sem = sem("gp")
        out_sem = sem("outd")

        def dma_in(eng, side_idx, u):
            img, off, w = units[u]
            side = (pred, targ)[side_idx]
            s_sem = (inp_sem, int_sem)[side_idx]
            src = side[img, :, :, off : off + w].rearrange("c p f -> p c f")
            eng.dma_start(
                out=inb[u % 2][side_idx][:, : C * w].rearrange(
                    "p (c f) -> p c f", c=C
                ),
                in_=src,
            ).then_inc(s_sem[u % 2], 16)

        block = ctx.enter_context(nc.Block(no_gpsimd_drain=True))

        @block.sync
        def _(sync):
            # even units ride the SP ring; odd units are issued from the ACT
            # stream (second HWDGE ring)
            for u in range(0, n_units, 2):
                if u >= 2:
                    # WAR inb[0][pred]: unit u-2's up STT (its last reader)
                    sync.wait_ge(u_sem, 2 * u - 3)
                dma_in(sync, 0, u)
                if u >= 2:
                    # WAR inb[0][targ]: unit u-2's ut STT (its last reader)
                    sync.wait_ge(u_sem, 2 * u - 2)
                dma_in(sync, 1, u)
            if n_units > 2:
                # bulk of partials early; only the last 2 units' cols remain
                sync.wait_ge(gp_sem, n_units - 2)
                sync.dma_start(
                    out=out[:, : 3 * (n_units - 2)],
                    in_=acc[:, : 3 * (n_units - 2)],
                ).then_inc(out_sem, 16)
            sync.wait_ge(gp_sem, n_units)
            # No out_sem wait after the final write: the block-exit drain
            # fences the HWDGE ring before NEFF completion.
            sync.dma_start(
                out=out[:, 3 * max(0, n_units - 2) :],
                in_=acc[:, 3 * max(0, n_units - 2) :],
            ).then_inc(out_sem, 16)

        @block.vector
        def _(vector):
            def accum(u):
                # max(Wp, Wt) elementwise, accum_out = per-partition sum
                w = units[u][2]
                vector.wait_ge(act_sem, 2 * (u + 1))
                vector.scalar_tensor_tensor(
                    scr[:, :w],
                    wb[u % 2][0][:, :w],
                    0.0,
                    wb[u % 2][1][:, :w],
                    op0=Alu.bypass,
                    op1=Alu.max,
                    accum_out=acc[:, 3 * u : 3 * u + 1],
                ).then_inc(gp_sem, 1)

            for u in range(n_units):
                w = units[u][2]
                for s in range(2):
                    vector.wait_ge((inp_sem, int_sem)[s][u % 2], 16 * (u // 2 + 1))
                    t = inb[u % 2][s]
                    vector.tensor_max(m1[:, :w], t[:, 0:w], t[:, w : 2 * w])
                    if u >= 2:
                        # WAR on ub[u%2][s]: ACT's W of unit u-2 (its reader)
                        vector.wait_ge(act_sem, 2 * (u - 1))
                    vector.scalar_tensor_tensor(
                        ub[u % 2][s][:, :w],
                        m1[:, :w],
                        0.0,
                        t[:, 2 * w : 3 * w],
                        op0=Alu.max,
                        op1=Alu.max,
                    ).then_inc(u_sem, 1)
                if u > 0:
                    accum(u - 1)
            accum(n_units - 1)

        @block.scalar
        def _(scalar):
            # odd units' input DMAs ride the ACT HWDGE ring. Unit 1 goes up
            # front (fresh slot, no WAR); unit n+2 is placed right after
            # W_{n,1}, whose u_sem wait (>= 2n+2 = 2(n+2)-2) already covers
            # both WAR conditions for slot (n+2)%2.
            if n_units > 1:
                dma_in(scalar, 0, 1)
                dma_in(scalar, 1, 1)
            for n in range(n_units):
                w = units[n][2]
                for s in range(2):
                    scalar.wait_ge(u_sem, 2 * n + s + 1)
                    if n >= 2:
                        # WAR on wb[n%2][s]: accum of unit n-2 (its reader)
                        scalar.wait_ge(gp_sem, n - 1)
                    scalar.activation(
                        wb[n % 2][s][:, :w],
                        ub[n % 2][s][:, :w],
                        Act.Relu,
                        bias=1.0,
                        scale=-1.0,
                        accum_out=acc[:, 3 * n + 1 + s : 3 * n + 2 + s],
                    ).then_inc(act_sem, 1)
                if n + 2 < n_units and (n + 2) % 2 == 1:
                    dma_in(scalar, 0, n + 2)
                    dma_in(scalar, 1, n + 2)

        # Skip the Block-exit all-engine barrier (~4.3us): every cross-engine
        # dependency is semaphore-gated and the per-engine exit drains
        # (no_gpsimd_drain path) still fence the DMA rings, so engines may
        # halt independently — NEFF completion waits for all engines anyway.
        nc.all_engine_barrier = lambda *a, **k: None

    del nc.all_engine_barrier  # restore class method
    return nc


_program = None


def _get_program():
    global _program
    if _program is None:
        _program = _build_program()
    return _program


def _finish(partials_list):
    """partials_list: per-core [P, 3*n_units] f32 with cols per unit:
    [sum max(Wp,Wt), sum Wp, sum Wt].
    sum|Vp-Vt| = 2*sum(max) - sum(Wp) - sum(Wt)."""
    total = np.float64(0.0)
    for p in partials_list:
        p = p.astype(np.float64)
        total += 2.0 * p[:, 0::3].sum() - p[:, 1::3].sum() - p[:, 2::3].sum()
    return np.array(total / N_PIX, dtype=np.float32)


def kernel(pred: np.ndarray, target: np.ndarray) -> np.ndarray:
    from concourse.bass_utils import run_bass_kernel_spmd

    nc = _get_program()
    pred = np.ascontiguousarray(pred, dtype=np.float32).reshape(
        N_CORES, N_IMG, C, P, F
    )
    target = np.ascontiguousarray(target, dtype=np.float32).reshape(
        N_CORES, N_IMG, C, P, F
    )
    in_maps = [{"pred": pred[i], "target": target[i]} for i in range(N_CORES)]
    res = run_bass_kernel_spmd(nc, in_maps, list(range(N_CORES)))
    return _finish([r["partials"] for r in res.results])

